# revision 1
# baseline (speedup 1.0000x reference)
"""Trainium2 Bass kernel for nn_Model_1245540515968 (gnn_message_passing).

Self-contained: kernel(**inputs) -> np.ndarray [128] per-structure energies.

Strategy (8 cores, graph/data parallel):
  - Shard by structure: core c owns structures [16c, 16c+16) and their atoms.
  - Edges assigned to the core owning their receiver; sorted by receiver and
    packed into 128-slot tiles spanning <= 8 receiver atoms each.
  - Algebraic restructure: with P[e,(s,r)] = onehot_species(sender)[s]*bess[e,r]
    (32 features) and sh_full[e,m] (16 real-spherical-harmonic cols), the
    per-atom invariant block A/Am collapses to
        U[(s,r), (atom,m)] = sum_e P[e,(s,r)] * sh_full[e,m] * mask[e,atom]
        Am_l[j, atom, m]   = W3_l[(s,r), j]^T U[(s,r), atom, m-block(l)]
    where W3_l = ((emb[s,:] (x) w_rad_l[r,:]) @ w_mix_l) * (2l+1)^-0.25 is a
    host-precomputed weight transform. U is built by one PE matmul per edge
    tile (contract over the 128 edge slots), i.e. gather/segment-sum become
    dense tensor-engine work. B = sum_lm Am^2, then B^2, species embedding,
    w_out contraction, and the per-structure segment-sum are small per-block
    matmul/vector ops.
  - Per-edge sender/receiver rows are host-pregathered from a packed
    [pos, onehot(species)] table (TRN2 SWDGE indirect DMA supports only one
    index per partition per instruction, making device-side per-edge gather
    instruction-bound).
"""
import os
import sys
from contextlib import ExitStack

import numpy as np

for _p in ("/opt/trn_rl_repo",):
    if _p not in sys.path and os.path.isdir(_p):
        sys.path.insert(0, _p)

import concourse.bass as bass
import concourse.tile as tile
from concourse import bacc, mybir
from concourse.bass import IndirectOffsetOnAxis
from concourse.bass_utils import run_bass_kernel_spmd

F32 = mybir.dt.float32
I32 = mybir.dt.int32
AX = mybir.AxisListType
OP = mybir.AluOpType
ACTF = mybir.ActivationFunctionType

N_ATOMS = 10000
N_EDGES = 200000
N_SPECIES = 4
N_RAD = 8
N_MAX = [8, 6, 4, 2]
K_MIX = 128
N_STRUCT = 128
CUTOFF = 5.0
N_CORES = 8
S_PER_CORE = N_STRUCT // N_CORES
P = 128
ASPAN = 8
TPB = 16  # tiles per atom-block (16 tiles * 8 slots = 128 atom slots)

# sh_full column order: [sh3 (7), l0-const (1), sh1 (3), sh2 (5)]
# chosen so (l3,l0) and (l1,l2) pair into contiguous 8-col m-groups.
M_OFF = {3: 0, 0: 7, 1: 8, 2: 11}
M_LEN = {0: 1, 1: 3, 2: 5, 3: 7}

C1 = 0.4886025119029199
C2A = 1.0925484305920792
C2B = 0.31539156525252005
C2C = 0.5462742152960396
C3A = 0.5900435899266435
C3B = 2.890611442640554
C3C = 0.4570457994644658
C3D = 0.3731763325901154
L0C = 0.28209479177387814


# ----------------------------------------------------------------------------
# Host preprocessing (index-derived structures + weight transforms)
# ----------------------------------------------------------------------------

def _preprocess(inputs):
    species = np.asarray(inputs['species'])
    senders = np.asarray(inputs['senders'])
    receivers = np.asarray(inputs['receivers'])
    batch_seg = np.asarray(inputs['batch_seg'])
    positions = np.asarray(inputs['positions'], dtype=np.float32)

    struct_starts = np.searchsorted(batch_seg, np.arange(N_STRUCT + 1))
    core_hi = struct_starts[(np.arange(N_CORES) + 1) * S_PER_CORE]

    edge_core = np.searchsorted(core_hi, receivers, side='right')
    cores = []
    for c in range(N_CORES):
        e_idx = np.nonzero(edge_core == c)[0]
        e_idx = e_idx[np.argsort(receivers[e_idx], kind='stable')]
        cores.append(dict(e_idx=e_idx, s_lo=c * S_PER_CORE))

    # tile packing
    for c in cores:
        rs = receivers[c['e_idx']]
        atoms, counts = np.unique(rs, return_counts=True)
        tiles = []
        cur, cur_e = [], 0
        ptr = 0
        for a, cnt in zip(atoms, counts):
            assert cnt <= P
            if len(cur) == ASPAN or cur_e + cnt > P:
                tiles.append(cur)
                cur, cur_e = [], 0
            cur.append((int(a), int(cnt), ptr))
            ptr += int(cnt)
            cur_e += int(cnt)
        if cur:
            tiles.append(cur)
        c['tiles'] = tiles
    nt_max = max(len(c['tiles']) for c in cores)
    NB = -(-nt_max // TPB)
    NT = NB * TPB

    for c in cores:
        send_idx = np.zeros((NT, P), np.int32)
        recv_idx = np.zeros((NT, P), np.int32)
        M = np.zeros((NT, P, ASPAN), np.float32)
        slot_atom = -np.ones((NB * P,), np.int64)
        e_idx = c['e_idx']
        for t, tile_atoms in enumerate(c['tiles']):
            s = 0
            for a_local, (a, cnt, ptr) in enumerate(tile_atoms):
                eds = e_idx[ptr:ptr + cnt]
                send_idx[t, s:s + cnt] = senders[eds]
                recv_idx[t, s:s + cnt] = receivers[eds]
                M[t, s:s + cnt, a_local] = 1.0
                slot_atom[t * ASPAN + a_local] = a
                s += cnt
        c['send_idx'] = send_idx
        c['recv_idx'] = recv_idx
        c['mmask'] = np.ascontiguousarray(
            M.transpose(1, 0, 2).reshape(P, NT * ASPAN))        # [128, NT*8]
        oh = np.zeros((N_SPECIES, NB * P), np.float32)
        S = np.zeros((NB, P, S_PER_CORE), np.float32)
        valid = slot_atom >= 0
        va = slot_atom[valid]
        oh[species[va], np.nonzero(valid)[0]] = 1.0
        S[np.nonzero(valid)[0] // P, np.nonzero(valid)[0] % P,
          batch_seg[va] - c['s_lo']] = 1.0
        c['slot_valid'] = valid
        c['slot_species'] = np.where(valid, np.where(valid, 0, 0) + (
            species[np.where(valid, slot_atom, 0)]), -1)
        c['sstr'] = np.ascontiguousarray(
            S.transpose(1, 0, 2).reshape(P, NB * S_PER_CORE))   # [128, NB*16]

    # weight transforms
    emb = np.asarray(inputs['emb'], np.float32)
    emb2 = np.asarray(inputs['emb2'], np.float32)
    w_out = np.asarray(inputs['w_out'], np.float32)
    scal = float(np.asarray(inputs['scaling'])[0])
    W3 = np.zeros((32, 4 * K_MIX), np.float32)
    for l in range(4):
        w_rad = np.asarray(inputs[f'w_rad{l}'], np.float32) * 0.5  # fcut 0.5 fold
        w_mix = np.asarray(inputs[f'w_mix{l}'], np.float32)
        n_l = N_MAX[l]
        W2 = np.einsum('sc,ri->sric', emb, w_rad).reshape(32, n_l * 16)
        w3 = (W2 @ w_mix) * (2 * l + 1) ** -0.25
        if l == 0:
            w3 = w3 * L0C  # l0 sh col is stored as constant L0C=1 -> fold here
        W3[:, l * K_MIX:(l + 1) * K_MIX] = w3
    E2s = (emb2 * w_out[None, :] * scal).astype(np.float32)     # [4, 128]
    cw = np.asarray(inputs['comp_weights'], np.float32)
    # fold composition term: per-structure sum of cw[species] (host weight-prep)
    cw_struct = np.zeros(N_STRUCT, np.float32)
    np.add.at(cw_struct, batch_seg, cw[species])

    gtab = np.concatenate([
        positions,
        (species[:, None] == np.arange(N_SPECIES)[None, :]).astype(np.float32),
        np.zeros((N_ATOMS, 1), np.float32)], axis=1)            # [N, 8]
    ones = np.ones((P, 1), np.float32)

    shared = dict(w3=W3, onesc=ones)
    in_maps = []
    for ci, c in enumerate(cores):
        m = dict(shared)
        m['einit'] = cw_struct[ci * S_PER_CORE:(ci + 1) * S_PER_CORE].reshape(
            S_PER_CORE, 1).copy()
        # host-side gather of per-slot sender/receiver rows (slot (t,p) ->
        # partition p, tile t). Device-side alternative (indirect DMA) costs
        # ~1 instruction per 128 indices on TRN2 SWDGE; host gather keeps the
        # Pool engine free.
        gs = gtab[c['send_idx']]            # [NT, 128, 8]
        gr = gtab[c['recv_idx']][:, :, :4]  # [NT, 128, 4]
        m['gsend'] = np.ascontiguousarray(
            gs.transpose(1, 0, 2).reshape(P, NT * 8))
        m['grecv'] = np.ascontiguousarray(
            gr.transpose(1, 0, 2).reshape(P, NT * 4))
        m['mmask'] = c['mmask']
        sp_slot = c['slot_species']
        e2full = np.where((sp_slot >= 0)[None, :],
                          E2s.T[:, np.clip(sp_slot, 0, 3)], 0.0).astype(np.float32)
        m['e2full'] = np.ascontiguousarray(e2full)          # [128 j, NB*128]
        m['sstr'] = c['sstr']
        in_maps.append(m)
    return in_maps, NT, NB


# ----------------------------------------------------------------------------
# Bass program
# ----------------------------------------------------------------------------

def _chunk_ranges(NB, n_chunks=2):
    """Split NB blocks into chunks (in tiles), block-aligned."""
    n_chunks = min(n_chunks, NB)
    base, rem = divmod(NB, n_chunks)
    out = []
    b0 = 0
    for i in range(n_chunks):
        nb = base + (1 if i < rem else 0)
        out.append((b0 * TPB, (b0 + nb) * TPB))
        b0 += nb
    return out


CFG = dict(nchunks=4, shexp_dve_mod=3, upool_bufs=2, u_tiles=8,
           epool_bufs=2, spool_bufs=2, small_psum=False, am_bufs=4,
           sq_dve_n=0, rec_eng='mix', geom_eng='pool', ucopy_eng='mix',
           pf_eng='pool', bess_eng='dve', ablate=())


def build_program(NT, NB, repeat=1, inputs_internal=False):
    cfg = CFG
    nc = bacc.Bacc("TRN2", target_bir_lowering=False, debug=False)
    kind = "Internal" if inputs_internal else "ExternalInput"

    gsend = nc.dram_tensor('gsend', [P, NT * 8], F32, kind=kind).ap()
    grecv = nc.dram_tensor('grecv', [P, NT * 4], F32, kind=kind).ap()
    mmask = nc.dram_tensor('mmask', [P, NT * ASPAN], F32, kind=kind).ap()
    w3 = nc.dram_tensor('w3', [32, 4 * K_MIX], F32, kind="ExternalInput").ap()
    einit = nc.dram_tensor('einit', [S_PER_CORE, 1], F32, kind="ExternalInput").ap()
    onesc = nc.dram_tensor('onesc', [P, 1], F32, kind="ExternalInput").ap()
    e2full = nc.dram_tensor('e2full', [P, NB * P], F32, kind=kind).ap()
    sstr = nc.dram_tensor('sstr', [P, NB * S_PER_CORE], F32, kind=kind).ap()
    eout = nc.dram_tensor('eout', [S_PER_CORE, 1], F32, kind="ExternalOutput").ap()

    with tile.TileContext(nc) as tc, ExitStack() as ctx:
        cpool = ctx.enter_context(tc.tile_pool(name="const", bufs=1))
        gpool = ctx.enter_context(tc.tile_pool(name="gath", bufs=1))
        tpool = ctx.enter_context(tc.tile_pool(name="temps", bufs=2))
        epool = ctx.enter_context(tc.tile_pool(name="shexp", bufs=cfg["epool_bufs"]))
        spool = ctx.enter_context(tc.tile_pool(name="sq", bufs=cfg["spool_bufs"]))
        upool = ctx.enter_context(tc.tile_pool(name="upsum", bufs=cfg["upool_bufs"], space="PSUM"))
        apool = ctx.enter_context(tc.tile_pool(name="ampsum", bufs=cfg["am_bufs"], space="PSUM"))
        if cfg["small_psum"]:
            smpool = ctx.enter_context(tc.tile_pool(name="smpsum", bufs=2, space="PSUM"))
        else:
            smpool = apool

        # ---- constants / per-core tables to SBUF ----
        w3_sb = cpool.tile([32, 4 * K_MIX], F32)
        nc.sync.dma_start(w3_sb[:], w3)
        ones_sb = cpool.tile([P, 1], F32)
        nc.sync.dma_start(ones_sb[:], onesc)
        e2_sb = cpool.tile([P, NB * P], F32)
        nc.sync.dma_start(e2_sb[:], e2full)
        sstr_sb = cpool.tile([P, NB * S_PER_CORE], F32)
        nc.sync.dma_start(sstr_sb[:], sstr)
        mm_sb = cpool.tile([P, NT, ASPAN], F32)
        nc.sync.dma_start(mm_sb[:], mmask.rearrange("p (t a) -> p t a", a=ASPAN))

        g_send = gpool.tile([P, NT, 8], F32)
        g_recv = gpool.tile([P, NT, 4], F32)
        sh = gpool.tile([P, NT, 16], F32)
        pf = gpool.tile([P, NT, 32], F32)
        u_sb = gpool.tile([32, TPB * P], F32)
        e_acc = cpool.tile([S_PER_CORE, 1], F32)
        nc.sync.dma_start(e_acc[:], einit)
        nc.gpsimd.memset(sh[:, :, M_OFF[0]:M_OFF[0] + 1], 1.0)
        bias_eps = cpool.tile([P, 1], F32)
        nc.gpsimd.memset(bias_eps[:], 1e-12)
        bias_hpi = cpool.tile([P, 1], F32)
        nc.gpsimd.memset(bias_hpi[:], float(np.pi / 2))
        bias_npi = cpool.tile([P, 1], F32)
        nc.gpsimd.memset(bias_npi[:], float(-np.pi))

        chunks = _chunk_ranges(NB, cfg["nchunks"])

        # ---- per-chunk loads + geometry ----
        for _rep in range(repeat):
          for (t0, t1) in chunks:
              T = t1 - t0
              for (d0, d1) in _chunk_ranges(NB, 4):
                  if d0 < t0 or d0 >= t1:
                      continue
                  nc.sync.dma_start(
                      g_send[:, d0:d1, :],
                      gsend.rearrange("p (t c) -> p t c", c=8)[:, d0:d1, :])
                  nc.sync.dma_start(
                      g_recv[:, d0:d1, :],
                      grecv.rearrange("p (t c) -> p t c", c=4)[:, d0:d1, :])

              GEO = nc.gpsimd if cfg['geom_eng'] == 'pool' else nc.vector
              rvec = tpool.tile([P, T, 3], F32, tag="rvec")
              nc.vector.tensor_tensor(rvec[:], g_recv[:, t0:t1, 0:3],
                                      g_send[:, t0:t1, 0:3], OP.subtract)
              sq3 = tpool.tile([P, T, 3], F32, tag="sq3")
              GEO.tensor_tensor(sq3[:], rvec[:], rvec[:], OP.mult)
              r2 = tpool.tile([P, T], F32, tag="r2")
              nc.vector.tensor_reduce(r2[:], sq3[:], axis=AX.X, op=OP.add)
              r = tpool.tile([P, T], F32, tag="r")
              nc.scalar.activation(r[:], r2[:], ACTF.Sqrt, bias=bias_eps[:])
              rinv = tpool.tile([P, T], F32, tag="rinv")
              nc.vector.reciprocal(rinv[:], r[:])
              xr = tpool.tile([P, T], F32, tag="xr")
              nc.vector.tensor_scalar(xr[:], r[:], 1.0 / CUTOFF, 1.0, OP.mult, OP.min)
              xrp = tpool.tile([P, T], F32, tag="xrp")
              GEO.tensor_scalar(xrp[:], xr[:], 1e-3, None, OP.add)
              xrinv = tpool.tile([P, T], F32, tag="xrinv")
              nc.vector.reciprocal(xrinv[:], xrp[:])
              u = tpool.tile([P, T, 3], F32, tag="u")
              nc.vector.tensor_tensor(
                  u[:], rvec[:], rinv[:].unsqueeze(2).broadcast_to([P, T, 3]), OP.mult)
              fc = tpool.tile([P, T], F32, tag="fc")
              nc.scalar.activation(fc[:], xr[:], ACTF.Sin, bias=bias_hpi[:], scale=float(-np.pi))
              # sin(n*pi*xr) via Chebyshev recurrence: s_{n+1} = 2*cos(t)*s_n - s_{n-1}
              sin_t = tpool.tile([P, T, N_RAD], F32, tag="sin_t")
              nc.scalar.activation(sin_t[:, :, 0:1],
                                   xr[:].unsqueeze(2), ACTF.Sin, scale=float(np.pi))
              cc = tpool.tile([P, T, 1], F32, tag="cc")
              _R = {'mix': None, 'dve': nc.vector, 'pool': nc.gpsimd}[cfg['rec_eng']]
              nc.gpsimd.tensor_scalar(cc[:], fc[:].unsqueeze(2), 2.0, None, OP.mult)
              nc.vector.tensor_tensor(sin_t[:, :, 1:2], cc[:], sin_t[:, :, 0:1], OP.mult)
              stmp = tpool.tile([P, T, 1], F32, tag="stmp")
              for n in range(3, N_RAD + 1):
                  eng = _R or (nc.gpsimd if n % 2 else nc.vector)
                  eng2 = _R or (nc.vector if n % 2 else nc.gpsimd)
                  eng.tensor_tensor(stmp[:], cc[:], sin_t[:, :, n - 2:n - 1], OP.mult)
                  eng2.tensor_tensor(sin_t[:, :, n - 1:n], stmp[:],
                                     sin_t[:, :, n - 3:n - 2], OP.subtract)
              fc1 = tpool.tile([P, T], F32, tag="fc1")
              GEO.tensor_scalar(fc1[:], fc[:], 1.0, None, OP.add)
              wfac = tpool.tile([P, T], F32, tag="wfac")
              nc.vector.tensor_tensor(wfac[:], fc1[:], xrinv[:], OP.mult)
              bess = tpool.tile([P, T, N_RAD], F32, tag="bess")
              _BE = nc.gpsimd if cfg['bess_eng'] == 'pool' else nc.vector
              _BE.tensor_tensor(
                  bess[:], sin_t[:], wfac[:].unsqueeze(2).broadcast_to([P, T, N_RAD]),
                  OP.mult)
              # P features: onehot (x) bess -> [P, T, 4, 8]
              _PE2 = nc.gpsimd if cfg['pf_eng'] == 'pool' else nc.vector
              _PE2.tensor_tensor(
                  pf[:, t0:t1, :].rearrange("p t (s r) -> p t s r", s=4),
                  g_send[:, t0:t1, 3:7].unsqueeze(3).broadcast_to([P, T, 4, N_RAD]),
                  bess[:].unsqueeze(2).broadcast_to([P, T, 4, N_RAD]), OP.mult)

              # spherical harmonics into sh[:, t0:t1, :]
              x = u[:, :, 0:1]
              y = u[:, :, 1:2]
              z = u[:, :, 2:3]
              shc = sh[:, t0:t1, :]
              # l1: cols M_OFF[1]+(y,z,x)
              nc.vector.tensor_scalar(shc[:, :, M_OFF[1]:M_OFF[1] + 2],
                                      u[:, :, 1:3], C1, None, OP.mult)
              nc.vector.tensor_scalar(shc[:, :, M_OFF[1] + 2:M_OFF[1] + 3],
                                      x, C1, None, OP.mult)
              pr2 = tpool.tile([P, T, 2], F32, tag="pr2")  # (xy, yz)
              GEO.tensor_tensor(pr2[:], u[:, :, 0:2], u[:, :, 1:3], OP.mult)
              przx = tpool.tile([P, T, 1], F32, tag="przx")  # xz
              GEO.tensor_tensor(przx[:], z, x, OP.mult)
              u2 = tpool.tile([P, T, 3], F32, tag="u2")
              GEO.tensor_tensor(u2[:], u[:], u[:], OP.mult)
              x2 = u2[:, :, 0:1]
              y2 = u2[:, :, 1:2]
              z2 = u2[:, :, 2:3]
              # l2 block at M_OFF[2]: [C2A*xy, C2A*yz, C2B*(3z2-1), C2A*xz, C2C*(x2-y2)]
              o2 = M_OFF[2]
              nc.vector.tensor_scalar(shc[:, :, o2:o2 + 2], pr2[:], C2A, None, OP.mult)
              nc.vector.tensor_scalar(shc[:, :, o2 + 2:o2 + 3], z2,
                                      3.0 * C2B, C2B, OP.mult, OP.subtract)
              nc.vector.tensor_scalar(shc[:, :, o2 + 3:o2 + 4], przx[:], C2A, None, OP.mult)
              xmy = tpool.tile([P, T, 1], F32, tag="xmy")
              GEO.tensor_tensor(xmy[:], x2, y2, OP.subtract)
              nc.vector.tensor_scalar(shc[:, :, o2 + 4:o2 + 5], xmy[:], C2C, None, OP.mult)
              # l3 block at M_OFF[3]=0:
              # [C3A*y*(3x2-y2), C3B*xy*z, C3C*y*(5z2-1), C3D*z*(5z2-3),
              #  C3C*x*(5z2-1), C3B2*z*(x2-y2), C3A*x*(x2-3y2)]
              s3a = tpool.tile([P, T, 1], F32, tag="s3a")
              GEO.tensor_scalar(s3a[:], x2, 3.0 * C3A, None, OP.mult)
              s3c = tpool.tile([P, T, 1], F32, tag="s3c")
              GEO.tensor_scalar(s3c[:], y2, C3A, None, OP.mult)
              s3b = tpool.tile([P, T, 1], F32, tag="s3b")
              GEO.tensor_tensor(s3b[:], s3a[:], s3c[:], OP.subtract)
              nc.vector.tensor_tensor(shc[:, :, 0:1], s3b[:], y, OP.mult)
              zc = tpool.tile([P, T, 1], F32, tag="zc")
              GEO.tensor_scalar(zc[:], z, C3B, None, OP.mult)
              nc.vector.tensor_tensor(shc[:, :, 1:2], pr2[:, :, 0:1], zc[:], OP.mult)
              t511 = tpool.tile([P, T, 1], F32, tag="t511")
              GEO.tensor_scalar(t511[:], z2, 5.0 * C3C, C3C, OP.mult, OP.subtract)
              nc.vector.tensor_tensor(shc[:, :, 2:3], y, t511[:], OP.mult)
              t533 = tpool.tile([P, T, 1], F32, tag="t533")
              GEO.tensor_scalar(t533[:], z2, 5.0 * C3D, 3.0 * C3D, OP.mult, OP.subtract)
              nc.vector.tensor_tensor(shc[:, :, 3:4], z, t533[:], OP.mult)
              nc.vector.tensor_tensor(shc[:, :, 4:5], x, t511[:], OP.mult)
              zc2 = tpool.tile([P, T, 1], F32, tag="zc2")
              GEO.tensor_scalar(zc2[:], z, 1.445305721320277, None, OP.mult)
              nc.vector.tensor_tensor(shc[:, :, 5:6], xmy[:], zc2[:], OP.mult)
              s4a = tpool.tile([P, T, 1], F32, tag="s4a")
              GEO.tensor_scalar(s4a[:], x2, C3A, None, OP.mult)
              s4b = tpool.tile([P, T, 1], F32, tag="s4b")
              GEO.tensor_scalar(s4b[:], y2, 3.0 * C3A, None, OP.mult)
              s4c = tpool.tile([P, T, 1], F32, tag="s4c")
              GEO.tensor_tensor(s4c[:], s4a[:], s4b[:], OP.subtract)
              nc.vector.tensor_tensor(shc[:, :, 6:7], s4c[:], x, OP.mult)

              # ---- per-block scatter + phase 2 for blocks in this chunk ----
              for b in range(t0 // TPB, t1 // TPB):
                  if 'blocks' in cfg['ablate']:
                      continue
                  sh_exp = epool.tile([P, TPB, ASPAN, 16], F32, tag="shexp")
                  eng = nc.vector if (b % cfg["shexp_dve_mod"] == 0) else nc.gpsimd
                  eng.tensor_tensor(
                      sh_exp[:],
                      sh[:, b * TPB:(b + 1) * TPB, :].unsqueeze(2)
                        .broadcast_to([P, TPB, ASPAN, 16]),
                      mm_sb[:, b * TPB:(b + 1) * TPB, :].unsqueeze(3)
                        .broadcast_to([P, TPB, ASPAN, 16]),
                      OP.mult)
                  UT = cfg["u_tiles"]
                  if 'scatter' in cfg['ablate']:
                      continue
                  for hb in range(TPB // UT):
                      u_ps = upool.tile([32, UT * P], F32, tag="ups", space="PSUM")
                      for tl in range(UT):
                          t = b * TPB + hb * UT + tl
                          nc.tensor.matmul(
                              u_ps[:, tl * P:(tl + 1) * P],
                              lhsT=pf[:, t, :],
                              rhs=sh_exp[:, hb * UT + tl, :, :].rearrange(
                                  "p a m -> p (a m)"),
                              start=True, stop=True)
                      _uc = cfg['ucopy_eng']
                      use_act = (_uc == 'act') or (_uc == 'mix' and hb % 2 == 0)
                      if use_act:
                          nc.scalar.copy(u_sb[:, hb * UT * P:(hb + 1) * UT * P], u_ps[:])
                      else:
                          nc.vector.tensor_copy(u_sb[:, hb * UT * P:(hb + 1) * UT * P], u_ps[:])

                  # phase 2: Am matmuls; psum col layout (ta, m8) interleaved
                  uv = u_sb[:].rearrange("q (ta m) -> q ta m", m=16)
                  sq = spool.tile([P, P, 16], F32, tag="sq")
                  if 'p2' in cfg['ablate']:
                      continue
                  for pair_i, (la, lb) in enumerate(((3, 0), (1, 2))):
                      moff_a, mlen_a = M_OFF[la], M_LEN[la]
                      moff_b, mlen_b = M_OFF[lb], M_LEN[lb]
                      ca = 64 * mlen_a
                      cb = 64 * mlen_b
                      for hh in range(2):
                          am = apool.tile([P, 512], F32, tag="am", space="PSUM")
                          ta0 = hh * 64
                          nc.tensor.matmul(
                              am[:, 0:ca],
                              lhsT=w3_sb[:, la * K_MIX:(la + 1) * K_MIX],
                              rhs=uv[:, ta0:ta0 + 64, moff_a:moff_a + mlen_a],
                              start=True, stop=True)
                          nc.tensor.matmul(
                              am[:, ca:ca + cb],
                              lhsT=w3_sb[:, lb * K_MIX:(lb + 1) * K_MIX],
                              rhs=uv[:, ta0:ta0 + 64, moff_b:moff_b + mlen_b],
                              start=True, stop=True)
                          # square PSUM -> sq SBUF at (ta, m) grid
                          sqi = pair_i * 4 + hh * 2
                          for (off, cols, moff, mlen) in (
                                  (0, ca, moff_a, mlen_a), (ca, cb, moff_b, mlen_b)):
                              if sqi % 8 < cfg['sq_dve_n']:
                                  nc.vector.tensor_tensor(
                                      sq[:, ta0:ta0 + 64, moff:moff + mlen],
                                      am[:, off:off + cols].rearrange(
                                          "p (ta m) -> p ta m", m=mlen),
                                      am[:, off:off + cols].rearrange(
                                          "p (ta m) -> p ta m", m=mlen), OP.mult)
                              else:
                                  nc.scalar.activation(
                                      sq[:, ta0:ta0 + 64, moff:moff + mlen],
                                      am[:, off:off + cols].rearrange(
                                          "p (ta m) -> p ta m", m=mlen),
                                      ACTF.Square)
                              sqi += 1
                  if 'sqred' in cfg['ablate']:
                      continue
                  B = spool.tile([P, P], F32, tag="B")
                  nc.vector.tensor_reduce(B[:], sq[:], axis=AX.X, op=OP.add)
                  B4 = spool.tile([P, P], F32, tag="B4")
                  nc.gpsimd.tensor_tensor(B4[:], B[:], B[:], OP.mult)
                  H = spool.tile([P, P], F32, tag="H")
                  nc.gpsimd.tensor_tensor(H[:], B4[:],
                                          e2_sb[:, b * P:(b + 1) * P], OP.mult)
                  at_ps = smpool.tile([P, 1], F32, tag="sm" if cfg["small_psum"] else "am", space="PSUM")
                  nc.tensor.matmul(at_ps[:], lhsT=H[:], rhs=ones_sb[:],
                                   start=True, stop=True)
                  at_sb = spool.tile([P, 1], F32, tag="at")
                  nc.scalar.copy(at_sb[:], at_ps[:])
                  eb_ps = smpool.tile([S_PER_CORE, 1], F32, tag="sm" if cfg["small_psum"] else "am", space="PSUM")
                  nc.tensor.matmul(
                      eb_ps[:], lhsT=sstr_sb[:, b * S_PER_CORE:(b + 1) * S_PER_CORE],
                      rhs=at_sb[:], start=True, stop=True)
                  nc.vector.tensor_tensor(e_acc[:], e_acc[:], eb_ps[:], OP.add)

        nc.sync.dma_start(eout, e_acc[:])

    nc.compile()
    return nc


_CACHE = {}


def _get_program(NT, NB):
    key = (NT, NB)
    if key not in _CACHE:
        _CACHE[key] = build_program(NT, NB)
    return _CACHE[key]


def run(inputs, trace=False, **kwargs):
    in_maps, NT, NB = _preprocess(inputs)
    nc = _get_program(NT, NB)
    res = run_bass_kernel_spmd(nc, in_maps, core_ids=list(range(N_CORES)),
                               trace=trace, **kwargs)
    out = np.concatenate([res.results[c]['eout'][:, 0] for c in range(N_CORES)])
    return out.astype(np.float32), res


def kernel(**inputs):
    out, _ = run(inputs)
    return out



# revision 41
# speedup vs baseline: 3.1652x; 3.1652x over previous
"""Trainium2 Bass kernel for nn_Model_1245540515968 (gnn_message_passing).

Self-contained: kernel(**inputs) -> np.ndarray [128] per-structure energies.

Strategy (8 cores, graph/data parallel):
  - Shard by structure: core c owns structures [16c, 16c+16) and their atoms.
  - Edges assigned to the core owning their receiver; best-fit-decreasing
    packed into 128-slot tiles spanning <= 6 receiver atoms each (degree-aware
    packing keeps atom-slot utilisation ~97%).
  - Algebraic restructure: with P[e,(s,r)] = onehot_species(sender)[s]*bess[e,r]
    (32 features) and sh_full[e,m] (16 real-spherical-harmonic cols),
        U[(s,r), (atom,m)] = sum_e P[e,(s,r)] * sh_full[e,m] * mask[e,atom]
        Am_l[j, atom, m]   = W3_l[(s,r), j]^T U[(s,r), atom, m-block(l)]
    with W3_l host-precomputed. U is one PE matmul per edge tile; B=sum Am^2.
  - fp16 on the matmul path (PE fp16 = 4x fp32); PSUM stays fp32; geometry
    fp32. A block's 16 tiles map to 4 PSUM partition-groups x 4 column-slots
    so U for a whole block is one [128, 384] PSUM bank and ONE psum->sbuf
    copy (w3 is host-replicated 4x across partition groups; the per-block
    atom index is ta = (g, q, a)).
  - Am matmuls write m-major [m, ta] grids; square + pair-sum fuse into a
    custom DVE op sq(a)+sq(b); the m-reduction is an fp16 packed add tree
    (DVE 2x mode); B^2 fuses into a second custom op sq(a+b).
  - Activation-table discipline: single full-NT geometry pass does the one
    Sqrt; Sin/Copy/Square then live in one table (2 loads total).
"""
import os
import sys
from contextlib import ExitStack

import numpy as np

F16NP = np.float16

for _p in ("/opt/trn_rl_repo",):
    if _p not in sys.path and os.path.isdir(_p):
        sys.path.insert(0, _p)

import concourse.bass as bass
import concourse.tile as tile
from concourse import bacc, mybir
from concourse.bass_utils import run_bass_kernel_spmd

F32 = mybir.dt.float32
F16 = mybir.dt.float16
AX = mybir.AxisListType
OP = mybir.AluOpType
ACTF = mybir.ActivationFunctionType

N_ATOMS = 10000
N_SPECIES = 4
N_RAD = 8
N_MAX = [8, 6, 4, 2]
K_MIX = 128
N_STRUCT = 128
CUTOFF = 5.0
N_CORES = 8
S_PER_CORE = N_STRUCT // N_CORES
P = 128
ASPAN = 6
TPB = 16                  # tiles per block
NG = 4                    # PSUM partition groups per block
NQ = TPB // NG            # column slots per group
SPB = TPB * ASPAN         # atom slots per block (96)
TAG = NQ * ASPAN          # ta per group (24)
UCOL = NQ * ASPAN * 16    # u_ps columns (384)

# sh_full column order: [sh3 (7), l0-const (1), sh1 (3), sh2 (5)]
M_OFF = {3: 0, 0: 7, 1: 8, 2: 11}
M_LEN = {0: 1, 1: 3, 2: 5, 3: 7}

C1 = 0.4886025119029199
C2A = 1.0925484305920792
C2B = 0.31539156525252005
C2C = 0.5462742152960396
C3A = 0.5900435899266435
C3B = 2.890611442640554
C3C = 0.4570457994644658
C3D = 0.3731763325901154
L0C = 0.28209479177387814


# ----------------------------------------------------------------------------
# Custom DVE ops
# ----------------------------------------------------------------------------

_OPS = {}


def _register_op(name, body_fn, ref):
    if name in _OPS:
        return _OPS[name]
    import concourse.dve_ops as dve_ops
    from concourse import dve_spec

    for op in dve_ops.OPS:
        if op.name == name:
            _OPS[name] = op
            return op
    spec = dve_spec.Spec(body=body_fn(dve_spec), reference=ref)
    op = dve_ops.DveOp(name, spec, subdim=False, uops_sha={})
    dve_ops.OPS.append(op)
    dve_ops._SUB_OPCODE_FOR_NAME[op.name] = (
        dve_ops._CUSTOM_DVE_ROW_BASE + len(dve_ops.OPS) - 1)
    dve_ops.CUSTOM_DVE_SPECS[op.name] = spec
    # self-pin table hashes (simple elementwise bodies inside lower() limits)
    for ver in ("v3", "v4"):
        try:
            op.compile(ver)
        except ValueError as e:
            import re
            mm = re.search(r"\(%s: (\w+)" % ver, str(e))
            if mm:
                op.uops_sha[ver] = mm.group(1)
                op.compile(ver)
            else:
                raise
    _OPS[name] = op
    return op


def _get_sq():
    return _register_op(
        "SQ_ONLY_ANT",
        lambda d: d.sq(d.Src0),
        lambda in0, in1, s0, s1, imm2: (in0 * in0).astype(np.float32))


def _get_sqacc():
    return _register_op(
        "SQACC_ANT",
        lambda d: d.sq(d.Src0) + d.Src1,
        lambda in0, in1, s0, s1, imm2: (in0 * in0 + in1).astype(np.float32))


def _get_sqadd():
    return _register_op(
        "SQ_OF_SUM_ANT",
        lambda d: d.sq(d.Src0 + d.Src1),
        lambda in0, in1, s0, s1, imm2: ((in0 + in1) ** 2).astype(np.float32))


# ----------------------------------------------------------------------------
# Host preprocessing
# ----------------------------------------------------------------------------

def _preprocess(inputs):
    species = np.asarray(inputs['species'])
    senders = np.asarray(inputs['senders'])
    receivers = np.asarray(inputs['receivers'])
    batch_seg = np.asarray(inputs['batch_seg'])
    positions = np.asarray(inputs['positions'], dtype=np.float32)

    struct_starts = np.searchsorted(batch_seg, np.arange(N_STRUCT + 1))
    core_hi = struct_starts[(np.arange(N_CORES) + 1) * S_PER_CORE]

    edge_core = np.searchsorted(core_hi, receivers, side='right')
    cores = []
    for c in range(N_CORES):
        e_idx = np.nonzero(edge_core == c)[0]
        e_idx = e_idx[np.argsort(receivers[e_idx], kind='stable')]
        cores.append(dict(e_idx=e_idx, s_lo=c * S_PER_CORE))

    # best-fit-decreasing tile packing: bins of <=128 edges, <=ASPAN atoms
    for c in cores:
        rs = receivers[c['e_idx']]
        atoms, counts = np.unique(rs, return_counts=True)
        ptrs = np.concatenate([[0], np.cumsum(counts)[:-1]])
        order = np.argsort(-counts, kind='stable')
        tiles = []
        loads = []
        for k in order:
            a, cnt, ptr = int(atoms[k]), int(counts[k]), int(ptrs[k])
            assert cnt <= P
            best, best_rem = -1, P + 1
            for i in range(len(tiles)):
                rem = P - loads[i] - cnt
                if len(tiles[i]) < ASPAN and rem >= 0 and rem < best_rem:
                    best, best_rem = i, rem
            if best < 0:
                tiles.append([])
                loads.append(0)
                best = len(tiles) - 1
            tiles[best].append((a, cnt, ptr))
            loads[best] += cnt
        c['tiles'] = tiles
    nt_max = max(len(c['tiles']) for c in cores)
    NB = -(-nt_max // TPB)
    NT = NB * TPB

    def ta_of(t, a_local):
        # block-grid atom index: tile t -> (group, slot) in its block
        b, tl = divmod(t, TPB)
        g, q = tl % NG, tl // NG
        return b * SPB + g * TAG + q * ASPAN + a_local

    for c in cores:
        send_idx = np.zeros((NT, P), np.int32)
        recv_idx = np.zeros((NT, P), np.int32)
        M = np.zeros((NT, P, ASPAN), np.float32)
        slot_atom = -np.ones((NT * ASPAN,), np.int64)
        e_idx = c['e_idx']
        for t, tile_atoms in enumerate(c['tiles']):
            s = 0
            for a_local, (a, cnt, ptr) in enumerate(tile_atoms):
                eds = e_idx[ptr:ptr + cnt]
                send_idx[t, s:s + cnt] = senders[eds]
                recv_idx[t, s:s + cnt] = receivers[eds]
                M[t, s:s + cnt, a_local] = 1.0
                slot_atom[ta_of(t, a_local)] = a
                s += cnt
        c['send_idx'] = send_idx
        c['recv_idx'] = recv_idx
        c['mmask'] = np.ascontiguousarray(
            M.transpose(1, 0, 2).reshape(P, NT * ASPAN)).astype(F16NP)
        S = np.zeros((NB, SPB, S_PER_CORE), np.float32)
        valid = slot_atom >= 0
        va = slot_atom[valid]
        vs = np.nonzero(valid)[0]
        S[vs // SPB, vs % SPB, batch_seg[va] - c['s_lo']] = 1.0
        c['slot_species'] = np.where(valid, species[np.where(valid, slot_atom, 0)], -1)
        c['sstr'] = np.ascontiguousarray(
            S.transpose(1, 0, 2).reshape(SPB, NB * S_PER_CORE)).astype(F16NP)

    # weight transforms
    emb = np.asarray(inputs['emb'], np.float32)
    emb2 = np.asarray(inputs['emb2'], np.float32)
    w_out = np.asarray(inputs['w_out'], np.float32)
    scal = float(np.asarray(inputs['scaling'])[0])
    W3 = np.zeros((32, 4 * K_MIX), np.float32)
    for l in range(4):
        w_rad = np.asarray(inputs[f'w_rad{l}'], np.float32) * 0.5  # fcut 0.5 fold
        w_mix = np.asarray(inputs[f'w_mix{l}'], np.float32)
        n_l = N_MAX[l]
        W2 = np.einsum('sc,ri->sric', emb, w_rad).reshape(32, n_l * 16)
        w3 = (W2 @ w_mix) * (2 * l + 1) ** -0.25
        if l == 0:
            w3 = w3 * L0C
        W3[:, l * K_MIX:(l + 1) * K_MIX] = w3
    W3R = np.tile(W3, (NG, 1))                                  # [128, 512]
    E2s = (emb2 * w_out[None, :] * scal).astype(np.float32)     # [4, 128]
    cw = np.asarray(inputs['comp_weights'], np.float32)
    cw_struct = np.zeros(N_STRUCT, np.float32)
    np.add.at(cw_struct, batch_seg, cw[species])

    oh_tab = (np.arange(N_SPECIES)[None, :] ==
              species[:, None]).astype(np.float32)              # [N, 4]
    ones = np.ones((P, 1), F16NP)

    shared = dict(w3=W3R.astype(F16NP), onesc=ones)
    in_maps = []
    for ci, c in enumerate(cores):
        m = dict(shared)
        m['einit'] = cw_struct[ci * S_PER_CORE:(ci + 1) * S_PER_CORE].reshape(
            S_PER_CORE, 1).copy()
        ps = positions[c['send_idx']]       # [NT, 128, 3]
        pr = positions[c['recv_idx']]
        oh = oh_tab[c['send_idx']]          # [NT, 128, 4]
        m['gposs'] = np.ascontiguousarray(
            ps.transpose(1, 0, 2).reshape(P, NT * 3))
        m['gposr'] = np.ascontiguousarray(
            pr.transpose(1, 0, 2).reshape(P, NT * 3))
        m['goh'] = np.ascontiguousarray(
            oh.transpose(1, 0, 2).reshape(P, NT * 4)).astype(F16NP)
        m['mmask'] = c['mmask']
        sp_slot = c['slot_species']
        e2full = np.where((sp_slot >= 0)[None, :],
                          E2s.T[:, np.clip(sp_slot, 0, 3)], 0.0)
        m['e2full'] = np.ascontiguousarray(e2full).astype(F16NP)  # [128, NB*SPB]
        m['sstr'] = c['sstr']
        in_maps.append(m)
    return in_maps, NT, NB


# ----------------------------------------------------------------------------
# Bass program
# ----------------------------------------------------------------------------

CFG = dict(nchunks=2, pf_pool_frac=0.5, shexp_pool=True,
           sqsum_act_n=0, sqadd_act=False, h_eng='dve', t4_eng='dve',
           geo_moves=('rvec', 'u', 'bess'), epool_bufs=2, spool_bufs=2,
           upool_bufs=2, apool_bufs=5, usb_bufs=3)


def build_program(NT, NB, repeat=1):
    sqop = _get_sq()
    sqacc = _get_sqacc()
    sqadd = _get_sqadd()
    cfg = dict(CFG)
    nc = bacc.Bacc("TRN2", target_bir_lowering=False, debug=False)

    gposs = nc.dram_tensor('gposs', [P, NT * 3], F32, kind="ExternalInput").ap()
    gposr = nc.dram_tensor('gposr', [P, NT * 3], F32, kind="ExternalInput").ap()
    goh = nc.dram_tensor('goh', [P, NT * 4], F16, kind="ExternalInput").ap()
    mmask = nc.dram_tensor('mmask', [P, NT * ASPAN], F16, kind="ExternalInput").ap()
    w3 = nc.dram_tensor('w3', [P, 4 * K_MIX], F16, kind="ExternalInput").ap()
    einit = nc.dram_tensor('einit', [S_PER_CORE, 1], F32, kind="ExternalInput").ap()
    onesc = nc.dram_tensor('onesc', [P, 1], F16, kind="ExternalInput").ap()
    e2full = nc.dram_tensor('e2full', [P, NB * SPB], F16, kind="ExternalInput").ap()
    sstr = nc.dram_tensor('sstr', [SPB, NB * S_PER_CORE], F16, kind="ExternalInput").ap()
    eout = nc.dram_tensor('eout', [S_PER_CORE, 1], F32, kind="ExternalOutput").ap()

    with tile.TileContext(nc) as tc, ExitStack() as ctx:
        cpool = ctx.enter_context(tc.tile_pool(name="const", bufs=1))
        gpool = ctx.enter_context(tc.tile_pool(name="gath", bufs=1))
        tpool = ctx.enter_context(tc.tile_pool(name="temps", bufs=2))
        epool = ctx.enter_context(tc.tile_pool(name="shexp", bufs=cfg["epool_bufs"]))
        spool = ctx.enter_context(tc.tile_pool(name="sq", bufs=cfg["spool_bufs"]))
        upool = ctx.enter_context(tc.tile_pool(name="upsum", bufs=cfg["upool_bufs"], space="PSUM"))
        uspool = ctx.enter_context(tc.tile_pool(name="usb", bufs=cfg["usb_bufs"]))
        apool = ctx.enter_context(tc.tile_pool(name="ampsum", bufs=cfg["apool_bufs"], space="PSUM"))
        bpool = ctx.enter_context(tc.tile_pool(name="ebpsum", bufs=1, space="PSUM"))
        atpool = ctx.enter_context(tc.tile_pool(name="atpsum", bufs=1, space="PSUM"))

        # ---- constants / per-core tables to SBUF ----
        w3_sb = cpool.tile([P, 4 * K_MIX], F16)
        nc.sync.dma_start(w3_sb[:], w3)
        ones_sb = cpool.tile([P, 1], F16)
        nc.sync.dma_start(ones_sb[:], onesc)
        e2_sb = cpool.tile([P, NB * SPB], F16)
        nc.sync.dma_start(e2_sb[:], e2full)
        sstr_sb = cpool.tile([SPB, NB * S_PER_CORE], F16)
        nc.sync.dma_start(sstr_sb[:], sstr)
        mm_sb = cpool.tile([P, NT, ASPAN], F16)
        nc.sync.dma_start(mm_sb[:], mmask.rearrange("p (t a) -> p t a", a=ASPAN))
        g_ps = gpool.tile([P, NT, 3], F32)
        nc.sync.dma_start(g_ps[:], gposs.rearrange("p (t c) -> p t c", c=3))
        g_pr = gpool.tile([P, NT, 3], F32)
        nc.sync.dma_start(g_pr[:], gposr.rearrange("p (t c) -> p t c", c=3))
        g_oh = gpool.tile([P, NT, 4], F16)
        nc.sync.dma_start(g_oh[:], goh.rearrange("p (t c) -> p t c", c=4))
        e_acc = cpool.tile([S_PER_CORE, 1], F32)
        nc.sync.dma_start(e_acc[:], einit)

        sh = gpool.tile([P, NT, 16], F16)
        pf = gpool.tile([P, NT, 32], F16)
        nc.gpsimd.memset(sh[:, :, M_OFF[0]:M_OFF[0] + 1], 1.0)
        bias_eps = cpool.tile([P, 1], F32)
        nc.gpsimd.memset(bias_eps[:], 1e-12)
        bias_hpi = cpool.tile([P, 1], F32)
        nc.gpsimd.memset(bias_hpi[:], float(np.pi / 2))
        nchunks = min(cfg["nchunks"], NB)
        base, rem = divmod(NB, nchunks)
        chunks = []
        b0 = 0
        for i in range(nchunks):
            nb = base + (1 if i < rem else 0)
            chunks.append((b0 * TPB, (b0 + nb) * TPB))
            b0 += nb

        for _rep in range(repeat):
          eb_ps = bpool.tile([S_PER_CORE, 1], F32, tag="eb", space="PSUM")
          # ---- phase A: full-NT edge geometry (one Sqrt -> 1 table load) ----
          rvec = tpool.tile([P, NT, 3], F32, tag="rvec")
          RV = nc.gpsimd if 'rvec' in cfg['geo_moves'] else nc.vector
          RV.tensor_tensor(rvec[:], g_pr[:], g_ps[:], OP.subtract)
          sq3 = tpool.tile([P, NT, 3], F32, tag="sq3")
          nc.gpsimd.tensor_tensor(sq3[:], rvec[:], rvec[:], OP.mult)
          r2 = tpool.tile([P, NT], F32, tag="r2")
          nc.vector.tensor_reduce(r2[:], sq3[:], axis=AX.X, op=OP.add)
          r = tpool.tile([P, NT], F32, tag="r")
          nc.scalar.activation(r[:], r2[:], ACTF.Sqrt, bias=bias_eps[:])
          rinv = tpool.tile([P, NT], F32, tag="rinv")
          nc.vector.reciprocal(rinv[:], r[:])
          xr = tpool.tile([P, NT], F32, tag="xr")
          nc.vector.tensor_scalar(xr[:], r[:], 1.0 / CUTOFF, 1.0, OP.mult, OP.min)
          xrp = tpool.tile([P, NT], F32, tag="xrp")
          nc.gpsimd.tensor_scalar(xrp[:], xr[:], 1e-3, None, OP.add)
          xrinv = tpool.tile([P, NT], F32, tag="xrinv")
          nc.vector.reciprocal(xrinv[:], xrp[:])
          u = tpool.tile([P, NT, 3], F32, tag="u")
          UE = nc.gpsimd if 'u' in cfg['geo_moves'] else nc.vector
          UE.tensor_tensor(
              u[:], rvec[:], rinv[:].unsqueeze(2).broadcast_to([P, NT, 3]), OP.mult)

          for ci, (t0, t1) in enumerate(chunks):
              T = t1 - t0
              GEO = nc.gpsimd
              # ---- phase B: radial basis + sh for this chunk ----
              fc = tpool.tile([P, T], F32, tag="fc")
              nc.scalar.activation(fc[:], xr[:, t0:t1], ACTF.Sin,
                                   bias=bias_hpi[:], scale=float(-np.pi))
              # sin(n*pi*xr) via Chebyshev: s_{n+1} = 2*cos(pi*xr)*s_n - s_{n-1}
              sin_t = tpool.tile([P, T, N_RAD], F32, tag="sin_t")
              nc.scalar.activation(sin_t[:, :, 0:1],
                                   xr[:, t0:t1].unsqueeze(2), ACTF.Sin,
                                   scale=float(np.pi))
              cc = tpool.tile([P, T, 1], F32, tag="cc")
              nc.gpsimd.tensor_scalar(cc[:], fc[:].unsqueeze(2), 2.0, None, OP.mult)
              nc.vector.tensor_tensor(sin_t[:, :, 1:2], cc[:], sin_t[:, :, 0:1],
                                      OP.mult)
              stmp = tpool.tile([P, T, 1], F32, tag="stmp")
              for n in range(3, N_RAD + 1):
                  eng = nc.gpsimd if n % 2 else nc.vector
                  eng2 = nc.vector if n % 2 else nc.gpsimd
                  eng.tensor_tensor(stmp[:], cc[:], sin_t[:, :, n - 2:n - 1], OP.mult)
                  eng2.tensor_tensor(sin_t[:, :, n - 1:n], stmp[:],
                                     sin_t[:, :, n - 3:n - 2], OP.subtract)
              fc1 = tpool.tile([P, T], F32, tag="fc1")
              GEO.tensor_scalar(fc1[:], fc[:], 1.0, None, OP.add)
              wfac = tpool.tile([P, T], F32, tag="wfac")
              nc.vector.tensor_tensor(wfac[:], fc1[:], xrinv[:, t0:t1], OP.mult)
              bess = tpool.tile([P, T, N_RAD], F32, tag="bess")
              BE = nc.gpsimd if 'bess' in cfg['geo_moves'] else nc.vector
              BE.tensor_tensor(
                  bess[:], sin_t[:], wfac[:].unsqueeze(2).broadcast_to([P, T, N_RAD]),
                  OP.mult)
              # P features: onehot (x) bess -> [P, T, 4, 8], split Pool/DVE
              tm = t0 + int(T * cfg["pf_pool_frac"])
              pfv = pf[:].rearrange("p t (s r) -> p t s r", s=4)
              for (eng, a0, a1) in ((nc.gpsimd, t0, tm), (nc.vector, tm, t1)):
                  if a1 > a0:
                      eng.tensor_tensor(
                          pfv[:, a0:a1],
                          g_oh[:, a0:a1, :].unsqueeze(3)
                              .broadcast_to([P, a1 - a0, 4, N_RAD]),
                          bess[:, a0 - t0:a1 - t0].unsqueeze(2)
                              .broadcast_to([P, a1 - a0, 4, N_RAD]), OP.mult)

              # spherical harmonics into sh[:, t0:t1, :]
              uc = u[:, t0:t1, :]
              x = uc[:, :, 0:1]
              y = uc[:, :, 1:2]
              z = uc[:, :, 2:3]
              shc = sh[:, t0:t1, :]
              nc.vector.tensor_scalar(shc[:, :, M_OFF[1]:M_OFF[1] + 3],
                                      uc[:], C1, None, OP.mult)
              pr2 = tpool.tile([P, T, 2], F32, tag="pr2")  # (xy, yz)
              GEO.tensor_tensor(pr2[:], uc[:, :, 0:2], uc[:, :, 1:3], OP.mult)
              przx = tpool.tile([P, T, 1], F32, tag="przx")  # xz
              GEO.tensor_tensor(przx[:], z, x, OP.mult)
              u2 = tpool.tile([P, T, 3], F32, tag="u2")
              GEO.tensor_tensor(u2[:], uc[:], uc[:], OP.mult)
              x2 = u2[:, :, 0:1]
              y2 = u2[:, :, 1:2]
              z2 = u2[:, :, 2:3]
              o2 = M_OFF[2]
              nc.vector.tensor_scalar(shc[:, :, o2:o2 + 2], pr2[:], C2A, None, OP.mult)
              nc.vector.tensor_scalar(shc[:, :, o2 + 2:o2 + 3], z2,
                                      3.0 * C2B, C2B, OP.mult, OP.subtract)
              nc.vector.tensor_scalar(shc[:, :, o2 + 3:o2 + 4], przx[:], C2A, None, OP.mult)
              xmy = tpool.tile([P, T, 1], F32, tag="xmy")
              GEO.tensor_tensor(xmy[:], x2, y2, OP.subtract)
              nc.vector.tensor_scalar(shc[:, :, o2 + 4:o2 + 5], xmy[:], C2C, None, OP.mult)
              s3a = tpool.tile([P, T, 1], F32, tag="s3a")
              GEO.tensor_scalar(s3a[:], x2, 3.0 * C3A, None, OP.mult)
              s3c = tpool.tile([P, T, 1], F32, tag="s3c")
              GEO.tensor_scalar(s3c[:], y2, C3A, None, OP.mult)
              s3b = tpool.tile([P, T, 1], F32, tag="s3b")
              GEO.tensor_tensor(s3b[:], s3a[:], s3c[:], OP.subtract)
              nc.vector.tensor_tensor(shc[:, :, 0:1], s3b[:], y, OP.mult)
              zc = tpool.tile([P, T, 1], F32, tag="zc")
              GEO.tensor_scalar(zc[:], z, C3B, None, OP.mult)
              nc.vector.tensor_tensor(shc[:, :, 1:2], pr2[:, :, 0:1], zc[:], OP.mult)
              t511 = tpool.tile([P, T, 1], F32, tag="t511")
              GEO.tensor_scalar(t511[:], z2, 5.0 * C3C, C3C, OP.mult, OP.subtract)
              nc.vector.tensor_tensor(shc[:, :, 2:3], y, t511[:], OP.mult)
              t533 = tpool.tile([P, T, 1], F32, tag="t533")
              GEO.tensor_scalar(t533[:], z2, 5.0 * C3D, 3.0 * C3D, OP.mult, OP.subtract)
              nc.vector.tensor_tensor(shc[:, :, 3:4], z, t533[:], OP.mult)
              nc.vector.tensor_tensor(shc[:, :, 4:5], x, t511[:], OP.mult)
              zc2 = tpool.tile([P, T, 1], F32, tag="zc2")
              GEO.tensor_scalar(zc2[:], z, 1.445305721320277, None, OP.mult)
              nc.vector.tensor_tensor(shc[:, :, 5:6], xmy[:], zc2[:], OP.mult)
              s4a = tpool.tile([P, T, 1], F32, tag="s4a")
              GEO.tensor_scalar(s4a[:], x2, C3A, None, OP.mult)
              s4b = tpool.tile([P, T, 1], F32, tag="s4b")
              GEO.tensor_scalar(s4b[:], y2, 3.0 * C3A, None, OP.mult)
              s4c = tpool.tile([P, T, 1], F32, tag="s4c")
              GEO.tensor_tensor(s4c[:], s4a[:], s4b[:], OP.subtract)
              nc.vector.tensor_tensor(shc[:, :, 6:7], s4c[:], x, OP.mult)

              # ---- per-block pipeline ----
              for b in range(t0 // TPB, t1 // TPB):
                  if 'blocks' in cfg['ablate']:
                      continue
                  sh_exp = epool.tile([P, TPB, ASPAN, 16], F16, tag="shexp")
                  SHE = nc.gpsimd if cfg["shexp_pool"] else nc.vector
                  SHE.tensor_tensor(
                      sh_exp[:],
                      sh[:, b * TPB:(b + 1) * TPB, :].unsqueeze(2)
                        .broadcast_to([P, TPB, ASPAN, 16]),
                      mm_sb[:, b * TPB:(b + 1) * TPB, :].unsqueeze(3)
                        .broadcast_to([P, TPB, ASPAN, 16]),
                      OP.mult)
                  # U: 16 tiles -> 4 partition groups x 4 column slots
                  u_ps = upool.tile([P, UCOL], F32, tag="ups", space="PSUM")
                  for tl in range(TPB):
                      g, q = tl % NG, tl // NG
                      nc.tensor.matmul(
                          u_ps[32 * g:32 * (g + 1), q * 96:(q + 1) * 96],
                          lhsT=pf[:, b * TPB + tl, :],
                          rhs=sh_exp[:, tl, :, :].rearrange("p a m -> p (a m)"),
                          start=True, stop=True, tile_position=(0, 32 * g))
                  u_sb = uspool.tile([P, UCOL], F16, tag="usb")
                  nc.scalar.copy(u_sb[:], u_ps[:])

                  if 'p2' in cfg['ablate']:
                      continue
                  # phase 2: Am matmuls, m-major [m, ta] grids; PSUM bank g
                  # holds group g only (one row tile-position per bank)
                  uv = u_sb[:].rearrange("p (ta m) -> p ta m", m=16)
                  t8 = spool.tile([P, NG, 200], F16, tag="t8")
                  am = apool.tile([P, NG, 512], F32, tag="am", space="PSUM")
                  for g in range(NG):
                      pg = slice(32 * g, 32 * (g + 1))
                      for (off, l) in ((0, 3), (7 * TAG, 0),
                                       (8 * TAG, 1), (11 * TAG, 2)):
                          ml = M_LEN[l]
                          mo = M_OFF[l]
                          nc.tensor.matmul(
                              am[:, g, off:off + ml * TAG],
                              lhsT=w3_sb[pg, l * K_MIX:(l + 1) * K_MIX],
                              rhs=uv[pg, :, mo:mo + ml]
                                  .rearrange("q ta m -> q m ta"),
                              start=True, stop=True,
                              tile_position=(32 * g, 0))
                  if 'after_am' in cfg['ablate']:
                      continue
                  t8f = t8[:, :, 0:8 * TAG]
                  if cfg["no_custom"]:
                      sq0 = spool.tile([P, NG, 8 * TAG], F16, tag="sq0")
                      sq1 = spool.tile([P, NG, 8 * TAG], F16, tag="sq1")
                      nc.scalar.activation(sq0[:], am[:, :, 0:8 * TAG],
                                           ACTF.Square)
                      nc.scalar.activation(sq1[:], am[:, :, 8 * TAG:16 * TAG],
                                           ACTF.Square)
                      nc.vector.tensor_tensor(t8f, sq0[:], sq1[:], OP.add)
                  else:
                      t8a = spool.tile([P, NG, 200], F16, tag="t8a")
                      nc.vector._custom_dve(sqop, out=t8a[:, :, 0:8 * TAG],
                                            in0=am[:, :, 0:8 * TAG])
                      nc.vector._custom_dve(
                          sqacc, out=t8f, in0=am[:, :, 8 * TAG:16 * TAG],
                          in1=t8a[:, :, 0:8 * TAG])

                  if 'after_am' in cfg['ablate'] or 'after_t8' in cfg['ablate']:
                      continue
                  t8d = t8[:, :, 0:8 * TAG].rearrange(
                      "p g (m ta) -> p g m ta", ta=TAG)
                  T4E = nc.gpsimd if cfg["t4_eng"] == 'pool' else nc.vector
                  t4 = spool.tile([P, NG, 4, TAG], F16, tag="t4")
                  T4E.tensor_tensor(t4[:], t8d[:, :, 0:4, :],
                                    t8d[:, :, 4:8, :], OP.add)
                  t2 = spool.tile([P, NG, 2, TAG], F16, tag="t2")
                  nc.vector.tensor_tensor(t2[:], t4[:, :, 0:2, :],
                                          t4[:, :, 2:4, :], OP.add)
                  B4 = spool.tile([P, NG, 32], F32, tag="B4")
                  if cfg["sqadd_act"] or cfg["no_custom"]:
                      Bt = spool.tile([P, SPB], F16, tag="Bt")
                      nc.vector.tensor_tensor(
                          Bt[:].rearrange("p (g ta) -> p g ta", g=NG),
                          t2[:, :, 0, :], t2[:, :, 1, :], OP.add)
                      nc.scalar.activation(
                          B4[:, :, 0:TAG],
                          Bt[:].rearrange("p (g ta) -> p g ta", g=NG),
                          ACTF.Square)
                  else:
                      nc.vector._custom_dve(
                          sqadd, out=B4[:, :, 0:TAG],
                          in0=t2[:, :, 0, :], in1=t2[:, :, 1, :])
                  HE = nc.gpsimd if cfg["h_eng"] == 'pool' else nc.vector
                  H = spool.tile([P, SPB], F16, tag="H")
                  HE.tensor_tensor(H[:].rearrange("p (g ta) -> p g ta", g=NG),
                                   B4[:, :, 0:TAG],
                                   e2_sb[:, b * SPB:(b + 1) * SPB]
                                   .rearrange("p (g ta) -> p g ta", g=NG),
                                   OP.mult)
                  if 'after_h' in cfg['ablate']:
                      continue
                  at_ps = atpool.tile([SPB, 1], F32, tag="atp", space="PSUM")
                  nc.tensor.matmul(at_ps[:], lhsT=H[:], rhs=ones_sb[:],
                                   start=True, stop=True)
                  at_sb = spool.tile([SPB, 1], F16, tag="at")
                  nc.scalar.copy(at_sb[:], at_ps[:])
                  nc.tensor.matmul(
                      eb_ps[:],
                      lhsT=sstr_sb[:, b * S_PER_CORE:(b + 1) * S_PER_CORE],
                      rhs=at_sb[:], start=(b == 0), stop=(b == NB - 1))
          if not (set(cfg['ablate']) & {'blocks', 'p2', 'after_am', 'after_t8', 'after_h'}):
              nc.vector.tensor_tensor(e_acc[:], e_acc[:], eb_ps[:], OP.add)

        nc.sync.dma_start(eout, e_acc[:])

    nc.compile()
    return nc


_CACHE = {}


def _get_program(NT, NB):
    key = (NT, NB)
    if key not in _CACHE:
        _CACHE[key] = build_program(NT, NB)
    return _CACHE[key]


def run(inputs, trace=False, **kwargs):
    in_maps, NT, NB = _preprocess(inputs)
    nc = _get_program(NT, NB)
    res = run_bass_kernel_spmd(nc, in_maps, core_ids=list(range(N_CORES)),
                               trace=trace, **kwargs)
    out = np.concatenate([res.results[c]['eout'][:, 0] for c in range(N_CORES)])
    return out.astype(np.float32), res


def kernel(**inputs):
    out, _ = run(inputs)
    return out
